# revision 11
# baseline (speedup 1.0000x reference)
"""Differentiable point-cloud renderer (bilinear splat) as a Bass/Tile kernel
for 8 Trainium2 NeuronCores.

Formulation: the bilinear scatter-add of point n into image[y, x] factorizes
as an outer product of 1-D "hat" functions:

    image[y, x] = sum_n f_n * hat(y - py_n) * hat(x - px_n)
    hat(t) = relu(1 - |t|)

so per batch the image is a single matmul  image = A^T @ B  with
    A[n, y] = f_n * hat(y - py_n)   (lhsT, fp16)
    B[n, x] = hat(x - px_n)         (rhs,  fp16)
contracting over points in K-tiles of 128 on the PE, accumulating in PSUM.

Hat construction (per K-tile, all standard ops — no custom DVE):
    t = iota05 - pE             signed; engine round-robined across
                                DVE(ts) / ACT(Identity+bias) / Pool(tt bcast)
    A3 = min(|ty|,1) * f        two wide 8-tile DVE ops (f16 pair-broadcast)
    B' = min(|tx|,1)            one wide 8-tile DVE op
With Aneg = A3 - f and Bneg = B' - 1, image = sum Aneg*Bneg expands into
P - C[y] - R[x] + S, all accumulated in one PSUM pass: C via a ones-column
appended to the rhs, R and S via the f column appended to lhsT (psum row 96),
and the rank-1 row correction folded back with a negated-ones matmul.

Sharding: pure data parallel, 16 batches per core. The 3 identical output
channels are replicated on the host (identical data).
"""

import functools
import sys

sys.path.insert(0, "/opt/trn_rl_repo")

import numpy as np

import concourse.bacc as bacc
import concourse.bass as bass
import concourse.mybir as mybir
import concourse.tile as tile
from concourse.bass_utils import run_bass_kernel_spmd
from concourse.masks import make_identity

B, N, H, W = 128, 16384, 224, 224
NCORES = 8
BPC = B // NCORES            # batches per core
KT = N // 128                # k-tiles (of 128 points) per batch
TW = 8                       # k-tiles per wide group
NG = KT // TW                # wide groups per batch
WB = W + 1                   # A block width incl. the f column
F32 = mybir.dt.float32
F16 = mybir.dt.float16
I32 = mybir.dt.int32
U16 = mybir.dt.uint16
AF = mybir.ActivationFunctionType
OP = mybir.AluOpType
AX = mybir.AxisListType
HPI = float(np.pi / 2)

# engine assignment for the per-tile |iota - p| ops, per position in a group
# D = vector(DVE), A = scalar(ACT), P = gpsimd(Pool)
EY = ["D", "A", "P", "D", "A", "D", "P", "A"]   # y-side   D3 A3 P2
EX = ["A", "D", "P", "A", "P", "A", "P", "D"]   # x-side   D2 A3 P3


def splat_kernel(tc, nc, pts_d, az_d, el_d, img_d):
    act = nc.scalar.activation
    ts_ = nc.vector.tensor_scalar
    tt_ = nc.vector.tensor_tensor
    stt = nc.vector.scalar_tensor_tensor
    tsp = nc.gpsimd.tensor_scalar
    ttp = nc.gpsimd.tensor_tensor
    stp = nc.gpsimd.scalar_tensor_tensor

    with (
        tc.tile_pool(name="const", bufs=1) as cpool,
        tc.tile_pool(name="persist", bufs=1) as ppool,
        tc.tile_pool(name="work", bufs=3) as wpool,
        tc.tile_pool(name="hat", bufs=3) as hpool,
        tc.tile_pool(name="mm", bufs=3) as mpool,
        tc.tile_pool(name="psum", bufs=2, space="PSUM") as pspool,
        tc.tile_pool(name="psmall", bufs=1, space="PSUM") as pspool2,
        tc.tile_pool(name="psb", bufs=2, space="PSUM") as pspoolb,
    ):
        # ---------------- constants ----------------
        ident = cpool.tile([128, 128], F32)
        make_identity(nc, ident[:])
        iota_i = cpool.tile([128, W], I32)
        nc.gpsimd.iota(iota_i[:], pattern=[[1, W]], base=0, channel_multiplier=0)
        iota_f = cpool.tile([128, W], F32)
        nc.vector.tensor_copy(iota_f[:], iota_i[:])
        # iota + 0.5 (so u = |iota05 - pE| with pE = p + 0.5)
        iota05f = cpool.tile([128, W], F32)
        ts_(iota05f[:], iota_f[:], 0.5, None, OP.add)
        iota05h = cpool.tile([128, W], F16)
        nc.vector.tensor_copy(iota05h[:], iota05f[:])
        ones_row = cpool.tile([1, 128], F32)
        nc.vector.memset(ones_row[:], 1.0)
        negones = cpool.tile([1, 128], F32)
        nc.vector.memset(negones[:], -1.0)

        # ---------------- rotation coefficients ----------------
        # pxE = 112*rx + 112 ; pyE = 112*ry + 112 ; rz unscaled
        # rx = x*ca + z*sa
        # ry = x*(se*sa) + y*ce + z*(-se*ca)
        # rz = x*(-ce*sa) + y*se + z*(ce*ca)
        az_sb = cpool.tile([1, BPC], F32)
        nc.sync.dma_start(out=az_sb[:], in_=az_d[None, :])
        el_sb = cpool.tile([1, BPC], F32)
        nc.sync.dma_start(out=el_sb[:], in_=el_d[None, :])
        Trow = cpool.tile([1, 4 * BPC], F32)   # ca sa ce se
        Rrow = cpool.tile([1, 8 * BPC], F32)   # scaled coeffs
        hpi = cpool.tile([1, 1], F32)
        nc.vector.memset(hpi[:], HPI)
        zero1 = cpool.tile([1, 1], F32)
        nc.vector.memset(zero1[:], 0.0)

        def tl(k):
            return Trow[:, k * BPC:(k + 1) * BPC]

        def sl(k):
            return Rrow[:, k * BPC:(k + 1) * BPC]

        # ScalarE Sin is only valid on [-pi, pi]; range-reduce args first.
        TPI = float(2 * np.pi)

        def sin_wrapped(out_ap, in_ap, shift):
            c = cpool.tile([1, BPC], F32, tag="sinw_c")
            if shift != 0.0:
                ts_(c[:], in_ap, shift, None, OP.add)
            else:
                nc.vector.tensor_copy(c[:], in_ap)
            m = cpool.tile([1, BPC], F32, tag="sinw_m")
            ts_(m[:], c[:], float(np.pi), None, OP.is_ge)
            w = cpool.tile([1, BPC], F32, tag="sinw_w")
            stt(w[:], m[:], -TPI, c[:], op0=OP.mult, op1=OP.add)
            act(out_ap, w[:], AF.Sin, bias=zero1[:])

        sin_wrapped(tl(0), az_sb[:], HPI)   # ca
        sin_wrapped(tl(1), az_sb[:], 0.0)   # sa
        sin_wrapped(tl(2), el_sb[:], HPI)   # ce
        sin_wrapped(tl(3), el_sb[:], 0.0)   # se

        ts_(sl(0), tl(0), 112.0, None, OP.mult)                     # 112*ca
        ts_(sl(1), tl(1), 112.0, None, OP.mult)                     # 112*sa
        stt(sl(2), tl(3), 112.0, tl(1), op0=OP.mult, op1=OP.mult)   # 112*se*sa
        ts_(sl(3), tl(2), 112.0, None, OP.mult)                     # 112*ce
        stt(sl(4), tl(3), -112.0, tl(0), op0=OP.mult, op1=OP.mult)  # -112*se*ca
        stt(sl(5), tl(2), -1.0, tl(1), op0=OP.mult, op1=OP.mult)    # -ce*sa
        nc.vector.tensor_copy(sl(6), tl(3))                         # se
        tt_(sl(7), tl(2), tl(0), op=OP.mult)                        # ce*ca

        # broadcast coeffs to all 128 partitions via ones-matmul
        Rp = pspool2.tile([128, 8 * BPC], F32, tag="ptmp")
        nc.tensor.matmul(out=Rp[:], lhsT=ones_row[:], rhs=Rrow[:],
                         start=True, stop=True)
        Rbc = cpool.tile([128, 8 * BPC], F32)
        nc.vector.tensor_copy(Rbc[:], Rp[:])

        def Rc(k, b):
            return Rbc[:, k * BPC + b:k * BPC + b + 1]

        # ---------------- phase 1: coordinates per batch ----------------
        # Layout: point index n = p*128 + q; partition p, k-tile q.
        px_all = ppool.tile([128, BPC * 128], F32)
        py_all = ppool.tile([128, BPC * 128], F32)
        rz_all = ppool.tile([128, BPC * 128], F32)
        # min in cols [0:BPC], max in cols [32:32+BPC]
        zred = ppool.tile([128, 64], F32)
        nc.vector.memset(zred[:], 0.0)

        for b in range(BPC):
            pts = wpool.tile([128, 384], F32)
            nc.sync.dma_start(
                out=pts[:],
                in_=pts_d[b].rearrange("(p q) c -> p (q c)", p=128),
            )
            pv = pts[:].rearrange("p (q c) -> p c q", c=3)
            x, y, z = pv[:, 0, :], pv[:, 1, :], pv[:, 2, :]

            pxb = px_all[:, b * 128:(b + 1) * 128]
            pyb = py_all[:, b * 128:(b + 1) * 128]
            rzb = rz_all[:, b * 128:(b + 1) * 128]

            # pxE chain on DVE
            t1 = wpool.tile([128, 128], F32)
            ts_(t1[:], x, Rc(0, b), 112.0, OP.mult, OP.add)
            stt(pxb, z, Rc(1, b), t1[:], op0=OP.mult, op1=OP.add)
            # pyE chain on DVE
            t2 = wpool.tile([128, 128], F32)
            ts_(t2[:], x, Rc(2, b), 112.0, OP.mult, OP.add)
            t3 = wpool.tile([128, 128], F32)
            stt(t3[:], y, Rc(3, b), t2[:], op0=OP.mult, op1=OP.add)
            stt(pyb, z, Rc(4, b), t3[:], op0=OP.mult, op1=OP.add)
            # rz chain on DVE
            t4 = wpool.tile([128, 128], F32)
            ts_(t4[:], x, Rc(5, b), None, OP.mult)
            t5 = wpool.tile([128, 128], F32)
            stt(t5[:], y, Rc(6, b), t4[:], op0=OP.mult, op1=OP.add)
            stt(rzb, z, Rc(7, b), t5[:], op0=OP.mult, op1=OP.add)

            nc.vector.tensor_reduce(zred[:, b:b + 1], rzb, axis=AX.X, op=OP.min)
            nc.vector.tensor_reduce(zred[:, 32 + b:32 + b + 1], rzb,
                                    axis=AX.X, op=OP.max)

        # ---------------- phase 1b: z min/max across partitions ----------------
        ztp = pspool2.tile([64, 128], F32, tag="ptmp")
        nc.tensor.transpose(out=ztp[:], in_=zred[:], identity=ident[:])
        zmm = cpool.tile([64, 1], F32)
        nc.vector.memset(zmm[:], 0.0)
        nc.vector.tensor_reduce(zmm[0:BPC, :], ztp[0:BPC, :], axis=AX.X, op=OP.min)
        nc.vector.tensor_reduce(zmm[32:32 + BPC, :], ztp[32:32 + BPC, :],
                                axis=AX.X, op=OP.max)
        zrp = pspool2.tile([1, 64], F32, tag="ptmp")
        nc.tensor.transpose(out=zrp[:], in_=zmm[:],
                            identity=ident[0:64, 0:64])
        zrow = cpool.tile([1, 64], F32)
        nc.vector.tensor_copy(zrow[:], zrp[:])
        zbp = pspool2.tile([128, 64], F32, tag="ptmp")
        nc.tensor.matmul(out=zbp[:], lhsT=ones_row[:], rhs=zrow[:],
                         start=True, stop=True)
        zbc = cpool.tile([128, 64], F32)
        nc.vector.tensor_copy(zbc[:], zbp[:])

        # feat = 0.3 + 0.7*(z - zmin)/(zmax - zmin + 1e-6) = z*inv07 + beta
        d_ = cpool.tile([128, BPC], F32)
        stt(d_[:], zbc[:, 32:32 + BPC], 1e-6, zbc[:, 0:BPC],
            op0=OP.add, op1=OP.subtract)
        rec = cpool.tile([128, BPC], F32)
        nc.vector.reciprocal(rec[:], d_[:])
        inv07 = cpool.tile([128, BPC], F32)
        ts_(inv07[:], rec[:], 0.7, None, OP.mult)
        tb = cpool.tile([128, BPC], F32)
        tt_(tb[:], zbc[:, 0:BPC], inv07[:], op=OP.mult)
        beta = cpool.tile([128, BPC], F32)
        ts_(beta[:], tb[:], -1.0, 0.3, OP.mult, OP.add)

        # ---------------- phase 2: hats + matmul per batch ----------------
        for b in range(BPC):
            pxE = px_all[:, b * 128:(b + 1) * 128]   # px + 0.5
            pyE = py_all[:, b * 128:(b + 1) * 128]
            rzb = rz_all[:, b * 128:(b + 1) * 128]

            # featm = (rz*inv07 + beta) * inbounds-mask   (DVE)
            feat = wpool.tile([128, 128], F32)
            ts_(feat[:], rzb, inv07[:, b:b + 1], beta[:, b:b + 1],
                OP.mult, OP.add)
            mx = wpool.tile([128, 128], F32)
            ts_(mx[:], pxE, 0.5, None, OP.is_ge)
            mx2 = wpool.tile([128, 128], F32)
            stt(mx2[:], pxE, 223.5, mx[:], op0=OP.is_lt, op1=OP.mult)
            my = wpool.tile([128, 128], F32)
            ts_(my[:], pyE, 0.5, None, OP.is_ge)
            my2 = wpool.tile([128, 128], F32)
            stt(my2[:], pyE, 223.5, my[:], op0=OP.is_lt, op1=OP.mult)
            fm = wpool.tile([128, 128], F32)
            tt_(fm[:], feat[:], mx2[:], op=OP.mult)
            featm = wpool.tile([128, 128], F32)
            tt_(featm[:], fm[:], my2[:], op=OP.mult)

            # negated coords for the ACT-path bias (ACT)
            negx = wpool.tile([128, 128], F32)
            act(negx[:], pxE, AF.Copy, scale=-1.0)
            negy = wpool.tile([128, 128], F32)
            act(negy[:], pyE, AF.Copy, scale=-1.0)

            # f16 pair-packed feat for the wide A finisher: f2[:,2q]=f2[:,2q+1]=featm[:,q]
            f2 = wpool.tile([128, 256], F16)
            f2v = f2[:].rearrange("p (q t) -> p q t", t=2)
            nc.gpsimd.tensor_copy(f2v[:, :, 0:1], featm[:].unsqueeze(2))
            nc.gpsimd.tensor_copy(f2v[:, :, 1:2], featm[:].unsqueeze(2))

            ps0 = pspool.tile([128, WB], F32)
            ps1 = pspool.tile([97, WB], F32)

            for g in range(NG):
                q0 = g * TW
                tyw = hpool.tile([128, TW * W], F16, tag="tyw")
                txw = hpool.tile([128, TW * W], F16, tag="txw")
                for j in range(TW):
                    q = q0 + j
                    blk = slice(j * W, (j + 1) * W)
                    e = EY[j]
                    if e == "D":
                        ts_(tyw[:, blk], iota05h[:], pyE[:, q:q + 1], None,
                            OP.subtract)
                    elif e == "A":
                        act(tyw[:, blk], iota05f[:], AF.Identity,
                            bias=negy[:, q:q + 1])
                    else:
                        ttp(tyw[:, blk], iota05f[:],
                            pyE[:, q:q + 1].broadcast_to([128, W]),
                            op=OP.subtract)
                    e = EX[j]
                    if e == "D":
                        ts_(txw[:, blk], iota05h[:], pxE[:, q:q + 1], None,
                            OP.subtract)
                    elif e == "A":
                        act(txw[:, blk], iota05f[:], AF.Identity,
                            bias=negx[:, q:q + 1])
                    else:
                        ttp(txw[:, blk], iota05f[:],
                            pxE[:, q:q + 1].broadcast_to([128, W]),
                            op=OP.subtract)

                # A blocks: [ A3 (224) | f (1) ] * 8, width 225 each
                Anegw = mpool.tile([128, TW * WB], F16, tag="An")
                Av = Anegw[:].rearrange("p (b w) -> p b w", w=WB)
                # f column (from f32 featm, cast to f16)
                nc.vector.tensor_copy(Av[:, :, W:W + 1],
                                      featm[:, q0:q0 + TW].unsqueeze(2))
                # clamp to [-1,1], then abs via sign-bit mask (uint16 view)
                cy = mpool.tile([128, TW * W], F16, tag="cy")
                ts_(cy[:], tyw[:], 1.0, -1.0, OP.min, OP.max)
                my = mpool.tile([128, TW * W], F16, tag="my")
                ts_(my[:].bitcast(U16), cy[:].bitcast(U16), 0x7FFF, None,
                    OP.bitwise_and)
                # A3 = min(|ty|,1) * f   (wide, f16; tt with pair-broadcast f)
                a_out = Av[:, :, 0:W].rearrange("p b (h t) -> p b h t", t=2)
                my_v = my[:].rearrange("p (b h t) -> p b h t", b=TW, t=2)
                f2b = (f2[:, 2 * q0:2 * (q0 + TW)]
                       .rearrange("p (b t) -> p b t", t=2)
                       .unsqueeze(2)
                       .broadcast_to([128, TW, W // 2, 2]))
                tt_(a_out, my_v, f2b, op=OP.mult)
                # B blocks: [ B' (224) | 1 ] * 8 ;  B' = min(|tx|,1)
                Bw = mpool.tile([128, TW * WB], F16, tag="Bn")
                Bv = Bw[:].rearrange("p (b w) -> p b w", w=WB)
                nc.gpsimd.memset(Bv[:, :, W:W + 1], 1.0)
                cx = mpool.tile([128, TW * W], F16, tag="cx")
                ts_(cx[:], txw[:], 1.0, -1.0, OP.min, OP.max)
                ts_(Bv[:, :, 0:W].bitcast(U16),
                    cx[:].rearrange("p (b w) -> p b w", w=W).bitcast(U16),
                    0x7FFF, None, OP.bitwise_and)

                for j in range(TW):
                    q = q0 + j
                    rhs = Bw[:, j * WB:(j + 1) * WB]
                    nc.tensor.matmul(out=ps0[:],
                                     lhsT=Anegw[:, j * WB:j * WB + 128],
                                     rhs=rhs,
                                     start=(q == 0), stop=False)
                    nc.tensor.matmul(out=ps1[:],
                                     lhsT=Anegw[:, j * WB + 128:(j + 1) * WB],
                                     rhs=rhs,
                                     start=(q == 0), stop=False)

            # corrections:  img = P - C[y] - R[x] + S
            #   C[y] = ps[:,224] (ones-col) ; R[x] = ps1[96,x] ; S = ps1[96,224]
            # fold -(R - S) into psum rows via a negated-ones matmul
            rs = wpool.tile([1, WB], F32, tag="rs")
            nc.vector.tensor_copy(rs[:], ps1[96:97, 0:WB])
            rrow2 = wpool.tile([1, W], F32, tag="rrow2")
            ts_(rrow2[:], rs[:, 0:W], rs[:, W:W + 1], None, OP.subtract)
            nc.tensor.matmul(out=ps0[:, 0:W], lhsT=negones[:], rhs=rrow2[:],
                             start=False, stop=True)
            nc.tensor.matmul(out=ps1[:, 0:W], lhsT=negones[:, 0:97],
                             rhs=rrow2[:], start=False, stop=True)
            # subtract the C column while draining
            c0 = wpool.tile([128, 1], F32, tag="c0")
            nc.vector.tensor_copy(c0[:], ps0[:, W:W + 1])
            out0 = wpool.tile([128, W], F32, tag="out0")
            ts_(out0[:], ps0[:, 0:W], c0[:], None, OP.subtract)
            negc1 = wpool.tile([128, 1], F32, tag="negc1")
            act(negc1[0:96, :], ps1[0:96, W:W + 1], AF.Copy, scale=-1.0)
            out1 = wpool.tile([128, W], F32, tag="out1")
            act(out1[0:96, :], ps1[0:96, 0:W], AF.Identity,
                bias=negc1[0:96, :])
            nc.sync.dma_start(out=img_d[b, 0:128, :], in_=out0[:])
            nc.sync.dma_start(out=img_d[b, 128:224, :], in_=out1[0:96, :])


@functools.lru_cache(maxsize=1)
def _get_compiled():
    nc = bacc.Bacc(
        "TRN2",
        target_bir_lowering=False,
        debug=False,
        enable_asserts=False,
        num_devices=NCORES,
    )
    pts_d = nc.dram_tensor("points", [BPC, N, 3], F32, kind="ExternalInput")
    az_d = nc.dram_tensor("azimuth", [BPC], F32, kind="ExternalInput")
    el_d = nc.dram_tensor("elevation", [BPC], F32, kind="ExternalInput")
    img_d = nc.dram_tensor("img", [BPC, H, W], F32, kind="ExternalOutput")
    with tile.TileContext(nc) as tc:
        splat_kernel(tc, nc, pts_d, az_d, el_d, img_d)
    nc.compile()
    return nc


def run_on_device(points, azimuth, elevation, trace=False, **kw):
    nc = _get_compiled()
    in_maps = []
    for i in range(NCORES):
        s = slice(i * BPC, (i + 1) * BPC)
        in_maps.append({
            "points": np.ascontiguousarray(points[s], dtype=np.float32),
            "azimuth": np.ascontiguousarray(azimuth[s], dtype=np.float32),
            "elevation": np.ascontiguousarray(elevation[s], dtype=np.float32),
        })
    return run_bass_kernel_spmd(nc, in_maps, list(range(NCORES)),
                                trace=trace, **kw)


def kernel(points, azimuth, elevation):
    res = run_on_device(points, azimuth, elevation)
    imgs = np.concatenate([res.results[i]["img"] for i in range(NCORES)], axis=0)
    out = np.empty((B, 3, H, W), dtype=np.float32)
    out[:] = imgs[:, None, :, :]
    return out


# revision 14
# speedup vs baseline: 1.6987x; 1.6987x over previous
"""Differentiable point-cloud renderer (bilinear splat) as a Bass/Tile kernel
for 8 Trainium2 NeuronCores.

Formulation: the bilinear scatter-add of point n into image[y, x] factorizes
as an outer product of 1-D "hat" functions:

    image[y, x] = sum_n featm_n * hat(y - py_n) * hat(x - px_n)
    hat(t) = relu(1 - |t|)

so per batch the image is a single matmul  image = A^T @ B  with
    A[n, y] = featm_n * hat(y - py_n)   (lhsT, fp16)
    B[n, x] = hat(x - px_n)             (rhs,  fp16)
contracting over points in K-tiles of 128 on the PE, accumulating in PSUM.

Sharding: pure data parallel, 16 batches per core. The 3 identical output
channels are replicated on the host (identical data).
"""

import functools
import sys

sys.path.insert(0, "/opt/trn_rl_repo")

import numpy as np

import concourse.bacc as bacc
import concourse.bass as bass
import concourse.mybir as mybir
import concourse.tile as tile
from concourse.bass_utils import run_bass_kernel_spmd
from concourse.masks import make_identity

from concourse import dve_ops as _dve_ops
from concourse.dve_spec import (
    C0 as _C0, C1 as _C1, C2 as _C2, Spec as _Spec, Src0 as _Src0,
    Zero as _Zero, lower as _dve_lower, maxx as _maxx, minn as _minn,
)
from concourse.dve_uop import DveOpSpec as _DveOpSpec


def _register_neghat():
    """Custom fused DVE op: out = min(|in0 - s0| + imm2, 0) * s1.
    With in0 = iota, s0 = p, s1 = f, imm2 = -1 this is -f*hat(j - p)
    in a single 1x DVE instruction."""
    for o in _dve_ops.OPS:
        if o.name == "NEGHAT_ANT":
            return o
    d = _Src0 - _C0
    spec = _Spec(
        body=_minn(_maxx(d, _Zero - d) + _C2, _Zero) * _C1,
        reference=lambda in0, in1, s0, s1, imm2: (
            np.minimum(np.abs(in0.astype(np.float32) - s0) + imm2, 0.0) * s1
        ).astype(np.float32),
    )
    row = _dve_ops._CUSTOM_DVE_ROW_BASE + len(_dve_ops.OPS)
    assert row < 0x20
    op = _dve_ops.DveOp("NEGHAT_ANT", spec, subdim=False, uops_sha={})
    for ver in ("v3", "v4"):
        try:
            u = _dve_lower(spec, ver=ver)
            op.uops_sha[ver] = _DveOpSpec(
                name="NEGHAT_ANT", opcode=row, uops=u, rd1_en=False
            ).sha(ver)
        except Exception:
            pass
    _dve_ops.OPS.append(op)
    _dve_ops._SUB_OPCODE_FOR_NAME["NEGHAT_ANT"] = row
    _dve_ops.CUSTOM_DVE_SPECS["NEGHAT_ANT"] = spec
    return op


NEGHAT = _register_neghat()


def _register_neghat_post():
    """out = min(in0 + imm2, 0) * s1 — 3-stage finisher (2x-eligible)."""
    for o in _dve_ops.OPS:
        if o.name == "NEGHATP_ANT":
            return o
    spec = _Spec(
        body=_minn(_Src0 + _C2, _Zero) * _C1,
        reference=lambda in0, in1, s0, s1, imm2: (
            np.minimum(in0.astype(np.float32) + imm2, 0.0) * s1
        ).astype(np.float32),
    )
    row = _dve_ops._CUSTOM_DVE_ROW_BASE + len(_dve_ops.OPS)
    assert row < 0x20
    op = _dve_ops.DveOp("NEGHATP_ANT", spec, subdim=False, uops_sha={},
                        perf_en={"v3": True, "v4": True})
    for ver in ("v3", "v4"):
        try:
            u = _dve_lower(spec, ver=ver)
            op.uops_sha[ver] = _DveOpSpec(
                name="NEGHATP_ANT", opcode=row, uops=u, rd1_en=False
            ).sha(ver)
        except Exception:
            pass
    _dve_ops.OPS.append(op)
    _dve_ops._SUB_OPCODE_FOR_NAME["NEGHATP_ANT"] = row
    _dve_ops.CUSTOM_DVE_SPECS["NEGHATP_ANT"] = spec
    return op


NEGHATP = _register_neghat_post()

B, N, H, W = 128, 16384, 224, 224
NCORES = 8
BPC = B // NCORES            # batches per core
KT = N // 128                # k-tiles (of 128 points) per batch
F32 = mybir.dt.float32
F16 = mybir.dt.float16
I32 = mybir.dt.int32
AF = mybir.ActivationFunctionType
OP = mybir.AluOpType
AX = mybir.AxisListType
HPI = float(np.pi / 2)


def splat_kernel(tc, nc, pts_d, az_d, el_d, img_d):
    act = nc.scalar.activation
    ts_ = nc.vector.tensor_scalar
    tt_ = nc.vector.tensor_tensor
    stt = nc.vector.scalar_tensor_tensor

    with (
        tc.tile_pool(name="const", bufs=1) as cpool,
        tc.tile_pool(name="persist", bufs=1) as ppool,
        tc.tile_pool(name="work", bufs=3) as wpool,
        tc.tile_pool(name="hat", bufs=4) as hpool,
        tc.tile_pool(name="psum", bufs=2, space="PSUM") as pspool,
        tc.tile_pool(name="psmall", bufs=1, space="PSUM") as pspool2,
    ):
        # ---------------- constants ----------------
        ident = cpool.tile([128, 128], F32)
        make_identity(nc, ident[:])
        iota_i = cpool.tile([128, W], I32)
        nc.gpsimd.iota(iota_i[:], pattern=[[1, W]], base=0, channel_multiplier=0)
        iota_f = cpool.tile([128, W], F32)
        nc.vector.tensor_copy(iota_f[:], iota_i[:])
        iota_h = cpool.tile([128, W], F16)
        nc.vector.tensor_copy(iota_h[:], iota_f[:])
        ones_row = cpool.tile([1, 128], F32)
        nc.vector.memset(ones_row[:], 1.0)
        I16 = mybir.dt.int16
        offs32 = cpool.tile([128, 16], I32)
        nc.gpsimd.iota(offs32[:], pattern=[[224, 8], [0, 2]], base=0,
                       channel_multiplier=0)
        offs16 = cpool.tile([128, 16], I16)
        nc.vector.tensor_copy(offs16[:], offs32[:])

        # ---------------- rotation coefficients ----------------
        # R = R_el @ R_az ;  rx = x*ca + z*sa
        #                    ry = x*(se*sa) + y*ce + z*(-se*ca)
        #                    rz = x*(-ce*sa) + y*se + z*(ce*ca)
        az_sb = cpool.tile([1, BPC], F32)
        nc.sync.dma_start(out=az_sb[:], in_=az_d[None, :])
        el_sb = cpool.tile([1, BPC], F32)
        nc.sync.dma_start(out=el_sb[:], in_=el_d[None, :])
        Rrow = cpool.tile([1, 8 * BPC], F32)
        hpi = cpool.tile([1, 1], F32)
        nc.vector.memset(hpi[:], HPI)
        zero1 = cpool.tile([1, 1], F32)
        nc.vector.memset(zero1[:], 0.0)

        def sl(k):
            return Rrow[:, k * BPC:(k + 1) * BPC]

        # ScalarE Sin is only valid on [-pi, pi]; range-reduce args first.
        TPI = float(2 * np.pi)

        def sin_wrapped(out_ap, in_ap, shift):
            c = cpool.tile([1, BPC], F32, tag="sinw_c")
            if shift != 0.0:
                ts_(c[:], in_ap, shift, None, OP.add)
            else:
                nc.vector.tensor_copy(c[:], in_ap)
            m = cpool.tile([1, BPC], F32, tag="sinw_m")
            ts_(m[:], c[:], float(np.pi), None, OP.is_ge)
            w = cpool.tile([1, BPC], F32, tag="sinw_w")
            stt(w[:], m[:], -TPI, c[:], op0=OP.mult, op1=OP.add)
            act(out_ap, w[:], AF.Sin, bias=zero1[:])

        sin_wrapped(sl(0), az_sb[:], HPI)   # ca
        sin_wrapped(sl(1), az_sb[:], 0.0)   # sa
        sin_wrapped(sl(3), el_sb[:], HPI)   # ce
        sin_wrapped(sl(6), el_sb[:], 0.0)   # se
        tt_(sl(2), sl(6), sl(1), op=OP.mult)                      # se*sa
        stt(sl(4), sl(6), -1.0, sl(0), op0=OP.mult, op1=OP.mult)  # -se*ca
        stt(sl(5), sl(3), -1.0, sl(1), op0=OP.mult, op1=OP.mult)  # -ce*sa
        tt_(sl(7), sl(3), sl(0), op=OP.mult)                      # ce*ca

        # broadcast R coeffs to all 128 partitions via ones-matmul
        Rp = pspool2.tile([128, 8 * BPC], F32, tag='ptmp')
        nc.tensor.matmul(out=Rp[:], lhsT=ones_row[:], rhs=Rrow[:],
                         start=True, stop=True)
        Rbc = cpool.tile([128, 8 * BPC], F32)
        nc.vector.tensor_copy(Rbc[:], Rp[:])

        def Rc(k, b):
            return Rbc[:, k * BPC + b:k * BPC + b + 1]

        # ---------------- phase 1: coordinates per batch ----------------
        # Layout: point index n = p*128 + q; partition p, k-tile q.
        # pxE = px + 0.5 = (rx+1)*112 ; pyE likewise.
        px_all = ppool.tile([128, BPC * 128], F32)
        py_all = ppool.tile([128, BPC * 128], F32)
        rz_all = ppool.tile([128, BPC * 128], F32)
        # min in cols [0:BPC], max in cols [32:32+BPC] (32-aligned partition
        # bases after the transpose)
        zred = ppool.tile([128, 64], F32)
        nc.vector.memset(zred[:], 0.0)

        for b in range(BPC):
            pts = wpool.tile([128, 384], F32)
            nc.sync.dma_start(
                out=pts[:],
                in_=pts_d[b].rearrange("(p q) c -> p (q c)", p=128),
            )
            pv = pts[:].rearrange("p (q c) -> p c q", c=3)
            x, y, z = pv[:, 0, :], pv[:, 1, :], pv[:, 2, :]

            pxb = px_all[:, b * 128:(b + 1) * 128]
            pyb = py_all[:, b * 128:(b + 1) * 128]
            rzb = rz_all[:, b * 128:(b + 1) * 128]

            t1 = wpool.tile([128, 128], F32)
            ts_(t1[:], x, Rc(0, b), None, OP.mult)
            rx = wpool.tile([128, 128], F32)
            stt(rx[:], z, Rc(1, b), t1[:], op0=OP.mult, op1=OP.add)
            ts_(pxb, rx[:], 1.0, 112.0, OP.add, OP.mult)

            t2 = wpool.tile([128, 128], F32)
            ts_(t2[:], x, Rc(2, b), None, OP.mult)
            t3 = wpool.tile([128, 128], F32)
            stt(t3[:], y, Rc(3, b), t2[:], op0=OP.mult, op1=OP.add)
            ry = wpool.tile([128, 128], F32)
            stt(ry[:], z, Rc(4, b), t3[:], op0=OP.mult, op1=OP.add)
            ts_(pyb, ry[:], 1.0, 112.0, OP.add, OP.mult)

            t4 = wpool.tile([128, 128], F32)
            ts_(t4[:], x, Rc(5, b), None, OP.mult)
            t5 = wpool.tile([128, 128], F32)
            stt(t5[:], y, Rc(6, b), t4[:], op0=OP.mult, op1=OP.add)
            stt(rzb, z, Rc(7, b), t5[:], op0=OP.mult, op1=OP.add)

            nc.vector.tensor_reduce(zred[:, b:b + 1], rzb, axis=AX.X, op=OP.min)
            nc.vector.tensor_reduce(zred[:, 32 + b:32 + b + 1], rzb,
                                    axis=AX.X, op=OP.max)

        # ---------------- phase 1b: z min/max across partitions ----------------
        ztp = pspool2.tile([64, 128], F32, tag='ptmp')
        nc.tensor.transpose(out=ztp[:], in_=zred[:], identity=ident[:])
        zmm = cpool.tile([64, 1], F32)
        nc.vector.memset(zmm[:], 0.0)
        nc.vector.tensor_reduce(zmm[0:BPC, :], ztp[0:BPC, :], axis=AX.X, op=OP.min)
        nc.vector.tensor_reduce(zmm[32:32 + BPC, :], ztp[32:32 + BPC, :],
                                axis=AX.X, op=OP.max)
        zrp = pspool2.tile([1, 64], F32, tag='ptmp')
        nc.tensor.transpose(out=zrp[:], in_=zmm[:],
                            identity=ident[0:64, 0:64])
        zrow = cpool.tile([1, 64], F32)
        nc.vector.tensor_copy(zrow[:], zrp[:])
        zbp = pspool2.tile([128, 64], F32, tag='ptmp')
        nc.tensor.matmul(out=zbp[:], lhsT=ones_row[:], rhs=zrow[:],
                         start=True, stop=True)
        zbc = cpool.tile([128, 64], F32)
        nc.vector.tensor_copy(zbc[:], zbp[:])

        # feat = 0.3 + 0.7*(z - zmin)/(zmax - zmin + 1e-6) = z*inv07 + beta
        d_ = cpool.tile([128, BPC], F32)
        stt(d_[:], zbc[:, 32:32 + BPC], 1e-6, zbc[:, 0:BPC],
            op0=OP.add, op1=OP.subtract)
        rec = cpool.tile([128, BPC], F32)
        nc.vector.reciprocal(rec[:], d_[:])
        inv07 = cpool.tile([128, BPC], F32)
        ts_(inv07[:], rec[:], 0.7, None, OP.mult)
        tb = cpool.tile([128, BPC], F32)
        tt_(tb[:], zbc[:, 0:BPC], inv07[:], op=OP.mult)
        beta = cpool.tile([128, BPC], F32)
        ts_(beta[:], tb[:], -1.0, 0.3, OP.mult, OP.add)

        # ---------------- phase 2: hats + matmul per batch ----------------
        # Negation trick: build Atn = -f*hat_y and Btn = -hat_x; the two
        # negations cancel in the matmul, so no fixup is needed.
        #   y-side: u' = ACT Abs(j*f - f*py) = f*|j-py| ;
        #           Atn = min(u'-f, 0) = -f*hat_y              (1 ACT + 1 DVE)
        #   x-side (DVE path): pn = min(j-1-px, 0), qn = min(px-1-j, 0)
        #           (2-src ts from shifted iota consts);
        #           Btn = max(pn, qn) = -hat_x  (8-tile-wide TT max)
        #   x-side (ACT path, to balance engines): ux = ACT Abs(j - px);
        #           Btn = min(ux-1, 0) (imm-chain)
        TW = 8                    # tiles per wide group
        NYACT = 96                # y-tiles on the ACT path per batch
        BF16 = mybir.dt.bfloat16
        for b in range(BPC):
            pxE = px_all[:, b * 128:(b + 1) * 128]   # px + 0.5
            pyE = py_all[:, b * 128:(b + 1) * 128]
            rzb = rz_all[:, b * 128:(b + 1) * 128]

            feat = wpool.tile([128, 128], F32)
            ts_(feat[:], rzb, inv07[:, b:b + 1], beta[:, b:b + 1],
                OP.mult, OP.add)
            # mask: px>=0 & px<223 & py>=0 & py<223   (pxE = px+0.5)
            mx = wpool.tile([128, 128], F32)
            ts_(mx[:], pxE, 0.5, None, OP.is_ge)
            mx2 = wpool.tile([128, 128], F32)
            stt(mx2[:], pxE, 223.5, mx[:], op0=OP.is_lt, op1=OP.mult)
            my = wpool.tile([128, 128], F32)
            ts_(my[:], pyE, 0.5, None, OP.is_ge)
            my2 = wpool.tile([128, 128], F32)
            stt(my2[:], pyE, 223.5, my[:], op0=OP.is_lt, op1=OP.mult)
            fm = wpool.tile([128, 128], F32)
            tt_(fm[:], feat[:], mx2[:], op=OP.mult)
            featm = wpool.tile([128, 128], F32)
            tt_(featm[:], fm[:], my2[:], op=OP.mult)
            pym05 = wpool.tile([128, 128], F32)   # py
            ts_(pym05[:], pyE, 0.5, 0.0, OP.subtract, OP.add)
            pyneg = wpool.tile([128, 128], F32)   # -py
            ts_(pyneg[:], pyE, -1.0, 0.5, OP.mult, OP.add)
            # x-side scatter prep: x0 = floor(px) clamped to [0,222], weights
            # (frx-1, -frx) = negated bilinear x-weights
            pxc = wpool.tile([128, 128], F32)
            ts_(pxc[:], pxE, 0.5, 222.99, OP.max, OP.min)
            pxf = wpool.tile([128, 128], F32)
            ts_(pxf[:], pxc[:], 0.5, None, OP.subtract)
            pxfm = wpool.tile([128, 128], F32)
            ts_(pxfm[:], pxf[:], 0.5, None, OP.subtract)
            xi0 = wpool.tile([128, 128], mybir.dt.int16)
            nc.vector.tensor_copy(xi0[:], pxfm[:])
            xi0f = wpool.tile([128, 128], F32)
            nc.vector.tensor_copy(xi0f[:], xi0[:])
            frx = wpool.tile([128, 128], F32)
            tt_(frx[:], pxf[:], xi0f[:], op=OP.subtract)
            w1n = wpool.tile([128, 128], F32)
            ts_(w1n[:], frx[:], 1.0, None, OP.subtract)
            w2n = wpool.tile([128, 128], F32)
            ts_(w2n[:], frx[:], -1.0, None, OP.mult)
            wpair = wpool.tile([128, 256], F16)
            wpv = wpair[:].rearrange("p (q t) -> p q t", t=2)
            nc.vector.tensor_copy(wpv[:, :, 0:1], w1n[:].unsqueeze(2))
            nc.vector.tensor_copy(wpv[:, :, 1:2], w2n[:].unsqueeze(2))
            pxf1 = wpool.tile([128, 128], F32)
            ts_(pxf1[:], pxf[:], 0.5, None, OP.add)
            xi1 = wpool.tile([128, 128], mybir.dt.int16)
            nc.vector.tensor_copy(xi1[:], pxf1[:])
            xpair = wpool.tile([128, 256], mybir.dt.int16)
            xpv = xpair[:].rearrange("p (q t) -> p q t", t=2)
            nc.vector.tensor_copy(xpv[:, :, 0:1], xi0[:].unsqueeze(2))
            nc.vector.tensor_copy(xpv[:, :, 1:2], xi1[:].unsqueeze(2))

            ps0 = pspool.tile([128, W], F32)
            ps1 = pspool.tile([128, W], F32)

            def ymm(q, btn_ap):
                At = hpool.tile([128, W], F16, tag="At")
                if q < KT - NYACT:
                    nc.vector._custom_dve(
                        NEGHAT, out=At[:], in0=iota_h[:],
                        s0=pym05[:, q:q + 1], s1=featm[:, q:q + 1], imm2=-1.0)
                else:
                    uy = hpool.tile([128, W], F16, tag="uy")
                    act(uy[:], iota_f[:], AF.Abs, bias=pyneg[:, q:q + 1])
                    nc.vector._custom_dve(
                        NEGHATP, out=At[:], in0=uy[:],
                        s0=0.0, s1=featm[:, q:q + 1], imm2=-1.0)
                nc.tensor.matmul(out=ps0[:], lhsT=At[:, 0:128], rhs=btn_ap,
                                 start=(q == 0), stop=(q == KT - 1))
                nc.tensor.matmul(out=ps1[0:96, :], lhsT=At[:, 128:224],
                                 rhs=btn_ap, start=(q == 0), stop=(q == KT - 1))

            for g in range(KT // TW):
                q0 = g * TW
                idxw = hpool.tile([128, 16], mybir.dt.int16, tag="idxw")
                tt_(idxw[:], xpair[:, 2 * q0:2 * q0 + 16], offs16[:],
                    op=OP.add)
                btnw = hpool.tile([128, TW * W], F16, tag="btnw")
                nc.gpsimd.local_scatter(btnw[:], wpair[:, 2 * q0:2 * q0 + 16],
                                        idxw[:], channels=128,
                                        num_elems=TW * W, num_idxs=16)
                for j in range(TW):
                    ymm(q0 + j, btnw[:, j * W:(j + 1) * W])

            out0 = wpool.tile([128, W], F32)
            nc.vector.tensor_copy(out0[:], ps0[:])
            out1 = wpool.tile([128, W], F32)
            nc.vector.tensor_copy(out1[0:96, :], ps1[0:96, :])
            nc.sync.dma_start(out=img_d[b, 0:128, :], in_=out0[:])
            nc.sync.dma_start(out=img_d[b, 128:224, :], in_=out1[0:96, :])


@functools.lru_cache(maxsize=1)
def _get_compiled():
    nc = bacc.Bacc(
        "TRN2",
        target_bir_lowering=False,
        debug=False,
        enable_asserts=False,
        num_devices=NCORES,
    )
    pts_d = nc.dram_tensor("points", [BPC, N, 3], F32, kind="ExternalInput")
    az_d = nc.dram_tensor("azimuth", [BPC], F32, kind="ExternalInput")
    el_d = nc.dram_tensor("elevation", [BPC], F32, kind="ExternalInput")
    img_d = nc.dram_tensor("img", [BPC, H, W], F32, kind="ExternalOutput")
    with tile.TileContext(nc) as tc:
        splat_kernel(tc, nc, pts_d, az_d, el_d, img_d)
    nc.compile()
    return nc


def run_on_device(points, azimuth, elevation, trace=False, **kw):
    nc = _get_compiled()
    in_maps = []
    for i in range(NCORES):
        s = slice(i * BPC, (i + 1) * BPC)
        in_maps.append({
            "points": np.ascontiguousarray(points[s], dtype=np.float32),
            "azimuth": np.ascontiguousarray(azimuth[s], dtype=np.float32),
            "elevation": np.ascontiguousarray(elevation[s], dtype=np.float32),
        })
    return run_bass_kernel_spmd(nc, in_maps, list(range(NCORES)),
                                trace=trace, **kw)


def kernel(points, azimuth, elevation):
    res = run_on_device(points, azimuth, elevation)
    imgs = np.concatenate([res.results[i]["img"] for i in range(NCORES)], axis=0)
    out = np.empty((B, 3, H, W), dtype=np.float32)
    out[:] = imgs[:, None, :, :]
    return out



# revision 15
# speedup vs baseline: 2.0605x; 1.2130x over previous
"""Differentiable point-cloud renderer (bilinear splat) as a Bass/Tile kernel
for 8 Trainium2 NeuronCores.

Formulation: the bilinear scatter-add of point n into image[y, x] factorizes
as an outer product of 1-D "hat" functions:

    image[y, x] = sum_n featm_n * hat(y - py_n) * hat(x - px_n)
    hat(t) = relu(1 - |t|)

so per batch the image is a single matmul  image = A^T @ B  with
    A[n, y] = featm_n * hat(y - py_n)   (lhsT, fp16)
    B[n, x] = hat(x - px_n)             (rhs,  fp16)
contracting over points in K-tiles of 128 on the PE, accumulating in PSUM.

Sharding: pure data parallel, 16 batches per core. The 3 identical output
channels are replicated on the host (identical data).
"""

import functools
import sys

sys.path.insert(0, "/opt/trn_rl_repo")

import numpy as np

import concourse.bacc as bacc
import concourse.bass as bass
import concourse.mybir as mybir
import concourse.tile as tile
from concourse.bass_utils import run_bass_kernel_spmd
from concourse.masks import make_identity

from concourse import dve_ops as _dve_ops
from concourse.dve_spec import (
    C0 as _C0, C1 as _C1, C2 as _C2, Spec as _Spec, Src0 as _Src0,
    Zero as _Zero, lower as _dve_lower, maxx as _maxx, minn as _minn,
)
from concourse.dve_uop import DveOpSpec as _DveOpSpec


def _register_neghat():
    """Custom fused DVE op: out = min(|in0 - s0| + imm2, 0) * s1.
    With in0 = iota, s0 = p, s1 = f, imm2 = -1 this is -f*hat(j - p)
    in a single 1x DVE instruction."""
    for o in _dve_ops.OPS:
        if o.name == "NEGHAT_ANT":
            return o
    d = _Src0 - _C0
    spec = _Spec(
        body=_minn(_maxx(d, _Zero - d) + _C2, _Zero) * _C1,
        reference=lambda in0, in1, s0, s1, imm2: (
            np.minimum(np.abs(in0.astype(np.float32) - s0) + imm2, 0.0) * s1
        ).astype(np.float32),
    )
    row = _dve_ops._CUSTOM_DVE_ROW_BASE + len(_dve_ops.OPS)
    assert row < 0x20
    op = _dve_ops.DveOp("NEGHAT_ANT", spec, subdim=False, uops_sha={})
    for ver in ("v3", "v4"):
        try:
            u = _dve_lower(spec, ver=ver)
            op.uops_sha[ver] = _DveOpSpec(
                name="NEGHAT_ANT", opcode=row, uops=u, rd1_en=False
            ).sha(ver)
        except Exception:
            pass
    _dve_ops.OPS.append(op)
    _dve_ops._SUB_OPCODE_FOR_NAME["NEGHAT_ANT"] = row
    _dve_ops.CUSTOM_DVE_SPECS["NEGHAT_ANT"] = spec
    return op


NEGHAT = _register_neghat()


def _register_neghat_post():
    """out = min(in0 + imm2, 0) * s1 — 3-stage finisher (2x-eligible)."""
    for o in _dve_ops.OPS:
        if o.name == "NEGHATP_ANT":
            return o
    spec = _Spec(
        body=_minn(_Src0 + _C2, _Zero) * _C1,
        reference=lambda in0, in1, s0, s1, imm2: (
            np.minimum(in0.astype(np.float32) + imm2, 0.0) * s1
        ).astype(np.float32),
    )
    row = _dve_ops._CUSTOM_DVE_ROW_BASE + len(_dve_ops.OPS)
    assert row < 0x20
    op = _dve_ops.DveOp("NEGHATP_ANT", spec, subdim=False, uops_sha={},
                        perf_en={"v3": True, "v4": True})
    for ver in ("v3", "v4"):
        try:
            u = _dve_lower(spec, ver=ver)
            op.uops_sha[ver] = _DveOpSpec(
                name="NEGHATP_ANT", opcode=row, uops=u, rd1_en=False
            ).sha(ver)
        except Exception:
            pass
    _dve_ops.OPS.append(op)
    _dve_ops._SUB_OPCODE_FOR_NAME["NEGHATP_ANT"] = row
    _dve_ops.CUSTOM_DVE_SPECS["NEGHATP_ANT"] = spec
    return op


NEGHATP = _register_neghat_post()

B, N, H, W = 128, 16384, 224, 224
NCORES = 8
BPC = B // NCORES            # batches per core
KT = N // 128                # k-tiles (of 128 points) per batch
F32 = mybir.dt.float32
F16 = mybir.dt.float16
I32 = mybir.dt.int32
AF = mybir.ActivationFunctionType
OP = mybir.AluOpType
AX = mybir.AxisListType
HPI = float(np.pi / 2)


def splat_kernel(tc, nc, pts_d, az_d, el_d, img_d):
    act = nc.scalar.activation
    ts_ = nc.vector.tensor_scalar
    tt_ = nc.vector.tensor_tensor
    stt = nc.vector.scalar_tensor_tensor

    with (
        tc.tile_pool(name="const", bufs=1) as cpool,
        tc.tile_pool(name="persist", bufs=1) as ppool,
        tc.tile_pool(name="work", bufs=3) as wpool,
        tc.tile_pool(name="hat", bufs=4) as hpool,
        tc.tile_pool(name="psum", bufs=2, space="PSUM") as pspool,
        tc.tile_pool(name="psmall", bufs=1, space="PSUM") as pspool2,
    ):
        # ---------------- constants ----------------
        ident = cpool.tile([128, 128], F32)
        make_identity(nc, ident[:])
        iota_i = cpool.tile([128, W], I32)
        nc.gpsimd.iota(iota_i[:], pattern=[[1, W]], base=0, channel_multiplier=0)
        iota_f = cpool.tile([128, W], F32)
        nc.vector.tensor_copy(iota_f[:], iota_i[:])
        iota_h = cpool.tile([128, W], F16)
        nc.vector.tensor_copy(iota_h[:], iota_f[:])
        ones_row = cpool.tile([1, 128], F32)
        nc.vector.memset(ones_row[:], 1.0)
        I16 = mybir.dt.int16
        offs32 = cpool.tile([128, 16], I32)
        nc.gpsimd.iota(offs32[:], pattern=[[224, 8], [0, 2]], base=0,
                       channel_multiplier=0)
        offs16 = cpool.tile([128, 16], I16)
        nc.vector.tensor_copy(offs16[:], offs32[:])

        # ---------------- rotation coefficients ----------------
        # R = R_el @ R_az ;  rx = x*ca + z*sa
        #                    ry = x*(se*sa) + y*ce + z*(-se*ca)
        #                    rz = x*(-ce*sa) + y*se + z*(ce*ca)
        az_sb = cpool.tile([1, BPC], F32)
        nc.sync.dma_start(out=az_sb[:], in_=az_d[None, :])
        el_sb = cpool.tile([1, BPC], F32)
        nc.sync.dma_start(out=el_sb[:], in_=el_d[None, :])
        Rrow = cpool.tile([1, 8 * BPC], F32)
        hpi = cpool.tile([1, 1], F32)
        nc.vector.memset(hpi[:], HPI)
        zero1 = cpool.tile([1, 1], F32)
        nc.vector.memset(zero1[:], 0.0)

        def sl(k):
            return Rrow[:, k * BPC:(k + 1) * BPC]

        # ScalarE Sin is only valid on [-pi, pi]; range-reduce args first.
        TPI = float(2 * np.pi)

        def sin_wrapped(out_ap, in_ap, shift):
            c = cpool.tile([1, BPC], F32, tag="sinw_c")
            if shift != 0.0:
                ts_(c[:], in_ap, shift, None, OP.add)
            else:
                nc.vector.tensor_copy(c[:], in_ap)
            m = cpool.tile([1, BPC], F32, tag="sinw_m")
            ts_(m[:], c[:], float(np.pi), None, OP.is_ge)
            w = cpool.tile([1, BPC], F32, tag="sinw_w")
            stt(w[:], m[:], -TPI, c[:], op0=OP.mult, op1=OP.add)
            act(out_ap, w[:], AF.Sin, bias=zero1[:])

        sin_wrapped(sl(0), az_sb[:], HPI)   # ca
        sin_wrapped(sl(1), az_sb[:], 0.0)   # sa
        sin_wrapped(sl(3), el_sb[:], HPI)   # ce
        sin_wrapped(sl(6), el_sb[:], 0.0)   # se
        tt_(sl(2), sl(6), sl(1), op=OP.mult)                      # se*sa
        stt(sl(4), sl(6), -1.0, sl(0), op0=OP.mult, op1=OP.mult)  # -se*ca
        stt(sl(5), sl(3), -1.0, sl(1), op0=OP.mult, op1=OP.mult)  # -ce*sa
        tt_(sl(7), sl(3), sl(0), op=OP.mult)                      # ce*ca

        # broadcast R coeffs to all 128 partitions via ones-matmul
        Rp = pspool2.tile([128, 8 * BPC], F32, tag='ptmp')
        nc.tensor.matmul(out=Rp[:], lhsT=ones_row[:], rhs=Rrow[:],
                         start=True, stop=True)
        Rbc = cpool.tile([128, 8 * BPC], F32)
        nc.vector.tensor_copy(Rbc[:], Rp[:])

        def Rc(k, b):
            return Rbc[:, k * BPC + b:k * BPC + b + 1]

        # ---------------- phase 1: coordinates per batch ----------------
        # Layout: point index n = p*128 + q; partition p, k-tile q.
        # pxE = px + 0.5 = (rx+1)*112 ; pyE likewise.
        px_all = ppool.tile([128, BPC * 128], F32)
        py_all = ppool.tile([128, BPC * 128], F32)
        rz_all = ppool.tile([128, BPC * 128], F32)
        # min in cols [0:BPC], max in cols [32:32+BPC] (32-aligned partition
        # bases after the transpose)
        zred = ppool.tile([128, 64], F32)
        nc.vector.memset(zred[:], 0.0)

        for b in range(BPC):
            pts = wpool.tile([128, 384], F32)
            nc.sync.dma_start(
                out=pts[:],
                in_=pts_d[b].rearrange("(p q) c -> p (q c)", p=128),
            )
            pv = pts[:].rearrange("p (q c) -> p c q", c=3)
            x, y, z = pv[:, 0, :], pv[:, 1, :], pv[:, 2, :]

            pxb = px_all[:, b * 128:(b + 1) * 128]
            pyb = py_all[:, b * 128:(b + 1) * 128]
            rzb = rz_all[:, b * 128:(b + 1) * 128]

            t1 = wpool.tile([128, 128], F32)
            ts_(t1[:], x, Rc(0, b), None, OP.mult)
            rx = wpool.tile([128, 128], F32)
            stt(rx[:], z, Rc(1, b), t1[:], op0=OP.mult, op1=OP.add)
            ts_(pxb, rx[:], 1.0, 112.0, OP.add, OP.mult)

            t2 = wpool.tile([128, 128], F32)
            ts_(t2[:], x, Rc(2, b), None, OP.mult)
            t3 = wpool.tile([128, 128], F32)
            stt(t3[:], y, Rc(3, b), t2[:], op0=OP.mult, op1=OP.add)
            ry = wpool.tile([128, 128], F32)
            stt(ry[:], z, Rc(4, b), t3[:], op0=OP.mult, op1=OP.add)
            ts_(pyb, ry[:], 1.0, 112.0, OP.add, OP.mult)

            t4 = wpool.tile([128, 128], F32)
            ts_(t4[:], x, Rc(5, b), None, OP.mult)
            t5 = wpool.tile([128, 128], F32)
            stt(t5[:], y, Rc(6, b), t4[:], op0=OP.mult, op1=OP.add)
            stt(rzb, z, Rc(7, b), t5[:], op0=OP.mult, op1=OP.add)

            nc.vector.tensor_reduce(zred[:, b:b + 1], rzb, axis=AX.X, op=OP.min)
            nc.vector.tensor_reduce(zred[:, 32 + b:32 + b + 1], rzb,
                                    axis=AX.X, op=OP.max)

        # ---------------- phase 1b: z min/max across partitions ----------------
        ztp = pspool2.tile([64, 128], F32, tag='ptmp')
        nc.tensor.transpose(out=ztp[:], in_=zred[:], identity=ident[:])
        zmm = cpool.tile([64, 1], F32)
        nc.vector.memset(zmm[:], 0.0)
        nc.vector.tensor_reduce(zmm[0:BPC, :], ztp[0:BPC, :], axis=AX.X, op=OP.min)
        nc.vector.tensor_reduce(zmm[32:32 + BPC, :], ztp[32:32 + BPC, :],
                                axis=AX.X, op=OP.max)
        zrp = pspool2.tile([1, 64], F32, tag='ptmp')
        nc.tensor.transpose(out=zrp[:], in_=zmm[:],
                            identity=ident[0:64, 0:64])
        zrow = cpool.tile([1, 64], F32)
        nc.vector.tensor_copy(zrow[:], zrp[:])
        zbp = pspool2.tile([128, 64], F32, tag='ptmp')
        nc.tensor.matmul(out=zbp[:], lhsT=ones_row[:], rhs=zrow[:],
                         start=True, stop=True)
        zbc = cpool.tile([128, 64], F32)
        nc.vector.tensor_copy(zbc[:], zbp[:])

        # feat = 0.3 + 0.7*(z - zmin)/(zmax - zmin + 1e-6) = z*inv07 + beta
        d_ = cpool.tile([128, BPC], F32)
        stt(d_[:], zbc[:, 32:32 + BPC], 1e-6, zbc[:, 0:BPC],
            op0=OP.add, op1=OP.subtract)
        rec = cpool.tile([128, BPC], F32)
        nc.vector.reciprocal(rec[:], d_[:])
        inv07 = cpool.tile([128, BPC], F32)
        ts_(inv07[:], rec[:], 0.7, None, OP.mult)
        tb = cpool.tile([128, BPC], F32)
        tt_(tb[:], zbc[:, 0:BPC], inv07[:], op=OP.mult)
        beta = cpool.tile([128, BPC], F32)
        ts_(beta[:], tb[:], -1.0, 0.3, OP.mult, OP.add)

        # ---------------- phase 2: hats + matmul per batch ----------------
        # Negation trick: build Atn = -f*hat_y and Btn = -hat_x; the two
        # negations cancel in the matmul, so no fixup is needed.
        #   y-side: u' = ACT Abs(j*f - f*py) = f*|j-py| ;
        #           Atn = min(u'-f, 0) = -f*hat_y              (1 ACT + 1 DVE)
        #   x-side (DVE path): pn = min(j-1-px, 0), qn = min(px-1-j, 0)
        #           (2-src ts from shifted iota consts);
        #           Btn = max(pn, qn) = -hat_x  (8-tile-wide TT max)
        #   x-side (ACT path, to balance engines): ux = ACT Abs(j - px);
        #           Btn = min(ux-1, 0) (imm-chain)
        TW = 8                    # tiles per wide group
        NYACT = 128               # y-tiles on the ACT path per batch
        BF16 = mybir.dt.bfloat16
        for b in range(BPC):
            pxE = px_all[:, b * 128:(b + 1) * 128]   # px + 0.5
            pyE = py_all[:, b * 128:(b + 1) * 128]
            rzb = rz_all[:, b * 128:(b + 1) * 128]

            feat = wpool.tile([128, 128], F32)
            ts_(feat[:], rzb, inv07[:, b:b + 1], beta[:, b:b + 1],
                OP.mult, OP.add)
            # mask: px>=0 & px<223 & py>=0 & py<223   (pxE = px+0.5)
            mx = wpool.tile([128, 128], F32)
            ts_(mx[:], pxE, 0.5, None, OP.is_ge)
            mx2 = wpool.tile([128, 128], F32)
            stt(mx2[:], pxE, 223.5, mx[:], op0=OP.is_lt, op1=OP.mult)
            my = wpool.tile([128, 128], F32)
            ts_(my[:], pyE, 0.5, None, OP.is_ge)
            my2 = wpool.tile([128, 128], F32)
            stt(my2[:], pyE, 223.5, my[:], op0=OP.is_lt, op1=OP.mult)
            fm = wpool.tile([128, 128], F32)
            tt_(fm[:], feat[:], mx2[:], op=OP.mult)
            featm = wpool.tile([128, 128], F32)
            tt_(featm[:], fm[:], my2[:], op=OP.mult)
            pym05 = wpool.tile([128, 128], F32)   # py
            ts_(pym05[:], pyE, 0.5, 0.0, OP.subtract, OP.add)
            pyneg = wpool.tile([128, 128], F32)   # -py
            ts_(pyneg[:], pyE, -1.0, 0.5, OP.mult, OP.add)
            # x-side scatter prep: x0 = floor(px) clamped to [0,222], weights
            # (frx-1, -frx) = negated bilinear x-weights
            # y-side scatter prep for the first SY tiles of each group:
            # weights (f*(fry-1), -f*fry) = negated f-scaled bilinear y-weights
            pyc = wpool.tile([128, 128], F32)
            ts_(pyc[:], pyE, 0.5, 222.99, OP.max, OP.min)
            pyf = wpool.tile([128, 128], F32)
            ts_(pyf[:], pyc[:], 0.5, None, OP.subtract)
            pyfm = wpool.tile([128, 128], F32)
            ts_(pyfm[:], pyf[:], 0.5, None, OP.subtract)
            yi0 = wpool.tile([128, 128], mybir.dt.int16)
            nc.vector.tensor_copy(yi0[:], pyfm[:])
            yi0f = wpool.tile([128, 128], F32)
            nc.vector.tensor_copy(yi0f[:], yi0[:])
            fry = wpool.tile([128, 128], F32)
            tt_(fry[:], pyf[:], yi0f[:], op=OP.subtract)
            wy1 = wpool.tile([128, 128], F32)
            stt(wy1[:], fry[:], 1.0, featm[:], op0=OP.subtract, op1=OP.mult)
            wy2 = wpool.tile([128, 128], F32)
            stt(wy2[:], fry[:], -1.0, featm[:], op0=OP.mult, op1=OP.mult)
            wypair = wpool.tile([128, 256], F16)
            wyv = wypair[:].rearrange("p (q t) -> p q t", t=2)
            nc.vector.tensor_copy(wyv[:, :, 0:1], wy1[:].unsqueeze(2))
            nc.vector.tensor_copy(wyv[:, :, 1:2], wy2[:].unsqueeze(2))
            pyf1 = wpool.tile([128, 128], F32)
            ts_(pyf1[:], pyf[:], 0.5, None, OP.add)
            yi1 = wpool.tile([128, 128], mybir.dt.int16)
            nc.vector.tensor_copy(yi1[:], pyf1[:])
            ypair = wpool.tile([128, 256], mybir.dt.int16)
            ypv = ypair[:].rearrange("p (q t) -> p q t", t=2)
            nc.vector.tensor_copy(ypv[:, :, 0:1], yi0[:].unsqueeze(2))
            nc.vector.tensor_copy(ypv[:, :, 1:2], yi1[:].unsqueeze(2))
            pxc = wpool.tile([128, 128], F32)
            ts_(pxc[:], pxE, 0.5, 222.99, OP.max, OP.min)
            pxf = wpool.tile([128, 128], F32)
            ts_(pxf[:], pxc[:], 0.5, None, OP.subtract)
            pxfm = wpool.tile([128, 128], F32)
            ts_(pxfm[:], pxf[:], 0.5, None, OP.subtract)
            xi0 = wpool.tile([128, 128], mybir.dt.int16)
            nc.vector.tensor_copy(xi0[:], pxfm[:])
            xi0f = wpool.tile([128, 128], F32)
            nc.vector.tensor_copy(xi0f[:], xi0[:])
            frx = wpool.tile([128, 128], F32)
            tt_(frx[:], pxf[:], xi0f[:], op=OP.subtract)
            w1n = wpool.tile([128, 128], F32)
            ts_(w1n[:], frx[:], 1.0, None, OP.subtract)
            w2n = wpool.tile([128, 128], F32)
            ts_(w2n[:], frx[:], -1.0, None, OP.mult)
            wpair = wpool.tile([128, 256], F16)
            wpv = wpair[:].rearrange("p (q t) -> p q t", t=2)
            nc.vector.tensor_copy(wpv[:, :, 0:1], w1n[:].unsqueeze(2))
            nc.vector.tensor_copy(wpv[:, :, 1:2], w2n[:].unsqueeze(2))
            pxf1 = wpool.tile([128, 128], F32)
            ts_(pxf1[:], pxf[:], 0.5, None, OP.add)
            xi1 = wpool.tile([128, 128], mybir.dt.int16)
            nc.vector.tensor_copy(xi1[:], pxf1[:])
            xpair = wpool.tile([128, 256], mybir.dt.int16)
            xpv = xpair[:].rearrange("p (q t) -> p q t", t=2)
            nc.vector.tensor_copy(xpv[:, :, 0:1], xi0[:].unsqueeze(2))
            nc.vector.tensor_copy(xpv[:, :, 1:2], xi1[:].unsqueeze(2))

            ps0 = pspool.tile([128, W], F32)
            ps1 = pspool.tile([128, W], F32)

            def ymm(q, btn_ap, at_ready=None):
                if at_ready is not None:
                    nc.tensor.matmul(out=ps0[:], lhsT=at_ready[:, 0:128],
                                     rhs=btn_ap,
                                     start=(q == 0), stop=(q == KT - 1))
                    nc.tensor.matmul(out=ps1[0:96, :],
                                     lhsT=at_ready[:, 128:224], rhs=btn_ap,
                                     start=(q == 0), stop=(q == KT - 1))
                    return
                At = hpool.tile([128, W], F16, tag="At")
                if q < KT - NYACT:
                    nc.vector._custom_dve(
                        NEGHAT, out=At[:], in0=iota_h[:],
                        s0=pym05[:, q:q + 1], s1=featm[:, q:q + 1], imm2=-1.0)
                else:
                    uy = hpool.tile([128, W], F16, tag="uy")
                    act(uy[:], iota_f[:], AF.Abs, bias=pyneg[:, q:q + 1])
                    nc.vector._custom_dve(
                        NEGHATP, out=At[:], in0=uy[:],
                        s0=0.0, s1=featm[:, q:q + 1], imm2=-1.0)
                nc.tensor.matmul(out=ps0[:], lhsT=At[:, 0:128], rhs=btn_ap,
                                 start=(q == 0), stop=(q == KT - 1))
                nc.tensor.matmul(out=ps1[0:96, :], lhsT=At[:, 128:224],
                                 rhs=btn_ap, start=(q == 0), stop=(q == KT - 1))

            SY = 4                    # y-tiles per group built by Pool scatter
            for g in range(KT // TW):
                q0 = g * TW
                idxw = hpool.tile([128, 16], mybir.dt.int16, tag="idxw")
                tt_(idxw[:], xpair[:, 2 * q0:2 * q0 + 16], offs16[:],
                    op=OP.add)
                btnw = hpool.tile([128, TW * W], F16, tag="btnw")
                nc.gpsimd.local_scatter(btnw[:], wpair[:, 2 * q0:2 * q0 + 16],
                                        idxw[:], channels=128,
                                        num_elems=TW * W, num_idxs=16)
                idxy = hpool.tile([128, 2 * SY], mybir.dt.int16, tag="idxy")
                tt_(idxy[:], ypair[:, 2 * q0:2 * q0 + 2 * SY],
                    offs16[:, 0:2 * SY], op=OP.add)
                atw = hpool.tile([128, SY * W], F16, tag="atw")
                nc.gpsimd.local_scatter(atw[:], wypair[:, 2 * q0:2 * q0 + 2 * SY],
                                        idxy[:], channels=128,
                                        num_elems=SY * W, num_idxs=2 * SY)
                for j in range(TW):
                    at = atw[:, j * W:(j + 1) * W] if j < SY else None
                    ymm(q0 + j, btnw[:, j * W:(j + 1) * W], at_ready=at)

            out0 = wpool.tile([128, W], F32)
            nc.vector.tensor_copy(out0[:], ps0[:])
            out1 = wpool.tile([128, W], F32)
            nc.vector.tensor_copy(out1[0:96, :], ps1[0:96, :])
            nc.sync.dma_start(out=img_d[b, 0:128, :], in_=out0[:])
            nc.sync.dma_start(out=img_d[b, 128:224, :], in_=out1[0:96, :])


@functools.lru_cache(maxsize=1)
def _get_compiled():
    nc = bacc.Bacc(
        "TRN2",
        target_bir_lowering=False,
        debug=False,
        enable_asserts=False,
        num_devices=NCORES,
    )
    pts_d = nc.dram_tensor("points", [BPC, N, 3], F32, kind="ExternalInput")
    az_d = nc.dram_tensor("azimuth", [BPC], F32, kind="ExternalInput")
    el_d = nc.dram_tensor("elevation", [BPC], F32, kind="ExternalInput")
    img_d = nc.dram_tensor("img", [BPC, H, W], F32, kind="ExternalOutput")
    with tile.TileContext(nc) as tc:
        splat_kernel(tc, nc, pts_d, az_d, el_d, img_d)
    nc.compile()
    return nc


def run_on_device(points, azimuth, elevation, trace=False, **kw):
    nc = _get_compiled()
    in_maps = []
    for i in range(NCORES):
        s = slice(i * BPC, (i + 1) * BPC)
        in_maps.append({
            "points": np.ascontiguousarray(points[s], dtype=np.float32),
            "azimuth": np.ascontiguousarray(azimuth[s], dtype=np.float32),
            "elevation": np.ascontiguousarray(elevation[s], dtype=np.float32),
        })
    return run_bass_kernel_spmd(nc, in_maps, list(range(NCORES)),
                                trace=trace, **kw)


def kernel(points, azimuth, elevation):
    res = run_on_device(points, azimuth, elevation)
    imgs = np.concatenate([res.results[i]["img"] for i in range(NCORES)], axis=0)
    out = np.empty((B, 3, H, W), dtype=np.float32)
    out[:] = imgs[:, None, :, :]
    return out



# revision 16
# speedup vs baseline: 2.0640x; 1.0017x over previous
"""Differentiable point-cloud renderer (bilinear splat) as a Bass/Tile kernel
for 8 Trainium2 NeuronCores.

Formulation: the bilinear scatter-add of point n into image[y, x] factorizes
as an outer product of 1-D "hat" functions:

    image[y, x] = sum_n featm_n * hat(y - py_n) * hat(x - px_n)
    hat(t) = relu(1 - |t|)

so per batch the image is a single matmul  image = A^T @ B  with
    A[n, y] = featm_n * hat(y - py_n)   (lhsT, fp16)
    B[n, x] = hat(x - px_n)             (rhs,  fp16)
contracting over points in K-tiles of 128 on the PE, accumulating in PSUM.

Sharding: pure data parallel, 16 batches per core. The 3 identical output
channels are replicated on the host (identical data).
"""

import functools
import sys

sys.path.insert(0, "/opt/trn_rl_repo")

import numpy as np

import concourse.bacc as bacc
import concourse.bass as bass
import concourse.mybir as mybir
import concourse.tile as tile
from concourse.bass_utils import run_bass_kernel_spmd
from concourse.masks import make_identity

from concourse import dve_ops as _dve_ops
from concourse.dve_spec import (
    C0 as _C0, C1 as _C1, C2 as _C2, Spec as _Spec, Src0 as _Src0,
    Zero as _Zero, lower as _dve_lower, maxx as _maxx, minn as _minn,
)
from concourse.dve_uop import DveOpSpec as _DveOpSpec


def _register_neghat():
    """Custom fused DVE op: out = min(|in0 - s0| + imm2, 0) * s1.
    With in0 = iota, s0 = p, s1 = f, imm2 = -1 this is -f*hat(j - p)
    in a single 1x DVE instruction."""
    for o in _dve_ops.OPS:
        if o.name == "NEGHAT_ANT":
            return o
    d = _Src0 - _C0
    spec = _Spec(
        body=_minn(_maxx(d, _Zero - d) + _C2, _Zero) * _C1,
        reference=lambda in0, in1, s0, s1, imm2: (
            np.minimum(np.abs(in0.astype(np.float32) - s0) + imm2, 0.0) * s1
        ).astype(np.float32),
    )
    row = _dve_ops._CUSTOM_DVE_ROW_BASE + len(_dve_ops.OPS)
    assert row < 0x20
    op = _dve_ops.DveOp("NEGHAT_ANT", spec, subdim=False, uops_sha={})
    for ver in ("v3", "v4"):
        try:
            u = _dve_lower(spec, ver=ver)
            op.uops_sha[ver] = _DveOpSpec(
                name="NEGHAT_ANT", opcode=row, uops=u, rd1_en=False
            ).sha(ver)
        except Exception:
            pass
    _dve_ops.OPS.append(op)
    _dve_ops._SUB_OPCODE_FOR_NAME["NEGHAT_ANT"] = row
    _dve_ops.CUSTOM_DVE_SPECS["NEGHAT_ANT"] = spec
    return op


NEGHAT = _register_neghat()


def _register_neghat_post():
    """out = min(in0 + imm2, 0) * s1 — 3-stage finisher (2x-eligible)."""
    for o in _dve_ops.OPS:
        if o.name == "NEGHATP_ANT":
            return o
    spec = _Spec(
        body=_minn(_Src0 + _C2, _Zero) * _C1,
        reference=lambda in0, in1, s0, s1, imm2: (
            np.minimum(in0.astype(np.float32) + imm2, 0.0) * s1
        ).astype(np.float32),
    )
    row = _dve_ops._CUSTOM_DVE_ROW_BASE + len(_dve_ops.OPS)
    assert row < 0x20
    op = _dve_ops.DveOp("NEGHATP_ANT", spec, subdim=False, uops_sha={},
                        perf_en={"v3": True, "v4": True})
    for ver in ("v3", "v4"):
        try:
            u = _dve_lower(spec, ver=ver)
            op.uops_sha[ver] = _DveOpSpec(
                name="NEGHATP_ANT", opcode=row, uops=u, rd1_en=False
            ).sha(ver)
        except Exception:
            pass
    _dve_ops.OPS.append(op)
    _dve_ops._SUB_OPCODE_FOR_NAME["NEGHATP_ANT"] = row
    _dve_ops.CUSTOM_DVE_SPECS["NEGHATP_ANT"] = spec
    return op


NEGHATP = _register_neghat_post()

B, N, H, W = 128, 16384, 224, 224
NCORES = 8
BPC = B // NCORES            # batches per core
KT = N // 128                # k-tiles (of 128 points) per batch
F32 = mybir.dt.float32
F16 = mybir.dt.float16
I32 = mybir.dt.int32
AF = mybir.ActivationFunctionType
OP = mybir.AluOpType
AX = mybir.AxisListType
HPI = float(np.pi / 2)


def splat_kernel(tc, nc, pts_d, az_d, el_d, img_d):
    act = nc.scalar.activation
    ts_ = nc.vector.tensor_scalar
    tt_ = nc.vector.tensor_tensor
    stt = nc.vector.scalar_tensor_tensor

    with (
        tc.tile_pool(name="const", bufs=1) as cpool,
        tc.tile_pool(name="persist", bufs=1) as ppool,
        tc.tile_pool(name="work", bufs=3) as wpool,
        tc.tile_pool(name="hat", bufs=5) as hpool,
        tc.tile_pool(name="psum", bufs=2, space="PSUM") as pspool,
        tc.tile_pool(name="psmall", bufs=1, space="PSUM") as pspool2,
    ):
        # ---------------- constants ----------------
        ident = cpool.tile([128, 128], F32)
        make_identity(nc, ident[:])
        iota_i = cpool.tile([128, W], I32)
        nc.gpsimd.iota(iota_i[:], pattern=[[1, W]], base=0, channel_multiplier=0)
        iota_f = cpool.tile([128, W], F32)
        nc.vector.tensor_copy(iota_f[:], iota_i[:])
        iota_h = cpool.tile([128, W], F16)
        nc.vector.tensor_copy(iota_h[:], iota_f[:])
        ones_row = cpool.tile([1, 128], F32)
        nc.vector.memset(ones_row[:], 1.0)
        I16 = mybir.dt.int16
        offs32 = cpool.tile([128, 256], I32)
        nc.gpsimd.iota(offs32[:], pattern=[[0, 16], [224, 8], [0, 2]], base=0,
                       channel_multiplier=0)
        offp = cpool.tile([128, 256], I16)
        nc.vector.tensor_copy(offp[:], offs32[:])

        # ---------------- rotation coefficients ----------------
        # R = R_el @ R_az ;  rx = x*ca + z*sa
        #                    ry = x*(se*sa) + y*ce + z*(-se*ca)
        #                    rz = x*(-ce*sa) + y*se + z*(ce*ca)
        az_sb = cpool.tile([1, BPC], F32)
        nc.sync.dma_start(out=az_sb[:], in_=az_d[None, :])
        el_sb = cpool.tile([1, BPC], F32)
        nc.sync.dma_start(out=el_sb[:], in_=el_d[None, :])
        Rrow = cpool.tile([1, 8 * BPC], F32)
        hpi = cpool.tile([1, 1], F32)
        nc.vector.memset(hpi[:], HPI)
        zero1 = cpool.tile([1, 1], F32)
        nc.vector.memset(zero1[:], 0.0)

        def sl(k):
            return Rrow[:, k * BPC:(k + 1) * BPC]

        # ScalarE Sin is only valid on [-pi, pi]; range-reduce args first.
        TPI = float(2 * np.pi)

        def sin_wrapped(out_ap, in_ap, shift):
            c = cpool.tile([1, BPC], F32, tag="sinw_c")
            if shift != 0.0:
                ts_(c[:], in_ap, shift, None, OP.add)
            else:
                nc.vector.tensor_copy(c[:], in_ap)
            m = cpool.tile([1, BPC], F32, tag="sinw_m")
            ts_(m[:], c[:], float(np.pi), None, OP.is_ge)
            w = cpool.tile([1, BPC], F32, tag="sinw_w")
            stt(w[:], m[:], -TPI, c[:], op0=OP.mult, op1=OP.add)
            act(out_ap, w[:], AF.Sin, bias=zero1[:])

        sin_wrapped(sl(0), az_sb[:], HPI)   # ca
        sin_wrapped(sl(1), az_sb[:], 0.0)   # sa
        sin_wrapped(sl(3), el_sb[:], HPI)   # ce
        sin_wrapped(sl(6), el_sb[:], 0.0)   # se
        tt_(sl(2), sl(6), sl(1), op=OP.mult)                      # se*sa
        stt(sl(4), sl(6), -1.0, sl(0), op0=OP.mult, op1=OP.mult)  # -se*ca
        stt(sl(5), sl(3), -1.0, sl(1), op0=OP.mult, op1=OP.mult)  # -ce*sa
        tt_(sl(7), sl(3), sl(0), op=OP.mult)                      # ce*ca

        # broadcast R coeffs to all 128 partitions via ones-matmul
        Rp = pspool2.tile([128, 8 * BPC], F32, tag='ptmp')
        nc.tensor.matmul(out=Rp[:], lhsT=ones_row[:], rhs=Rrow[:],
                         start=True, stop=True)
        Rbc = cpool.tile([128, 8 * BPC], F32)
        nc.vector.tensor_copy(Rbc[:], Rp[:])

        def Rc(k, b):
            return Rbc[:, k * BPC + b:k * BPC + b + 1]

        # ---------------- phase 1: coordinates per batch ----------------
        # Layout: point index n = p*128 + q; partition p, k-tile q.
        # pxE = px + 0.5 = (rx+1)*112 ; pyE likewise.
        px_all = ppool.tile([128, BPC * 128], F32)
        py_all = ppool.tile([128, BPC * 128], F32)
        rz_all = ppool.tile([128, BPC * 128], F32)
        # min in cols [0:BPC], max in cols [32:32+BPC] (32-aligned partition
        # bases after the transpose)
        zred = ppool.tile([128, 64], F32)
        nc.vector.memset(zred[:], 0.0)

        for b in range(BPC):
            pts = wpool.tile([128, 384], F32)
            nc.sync.dma_start(
                out=pts[:],
                in_=pts_d[b].rearrange("(p q) c -> p (q c)", p=128),
            )
            pv = pts[:].rearrange("p (q c) -> p c q", c=3)
            x, y, z = pv[:, 0, :], pv[:, 1, :], pv[:, 2, :]

            pxb = px_all[:, b * 128:(b + 1) * 128]
            pyb = py_all[:, b * 128:(b + 1) * 128]
            rzb = rz_all[:, b * 128:(b + 1) * 128]

            t1 = wpool.tile([128, 128], F32)
            ts_(t1[:], x, Rc(0, b), None, OP.mult)
            rx = wpool.tile([128, 128], F32)
            stt(rx[:], z, Rc(1, b), t1[:], op0=OP.mult, op1=OP.add)
            ts_(pxb, rx[:], 1.0, 112.0, OP.add, OP.mult)

            t2 = wpool.tile([128, 128], F32)
            ts_(t2[:], x, Rc(2, b), None, OP.mult)
            t3 = wpool.tile([128, 128], F32)
            stt(t3[:], y, Rc(3, b), t2[:], op0=OP.mult, op1=OP.add)
            ry = wpool.tile([128, 128], F32)
            stt(ry[:], z, Rc(4, b), t3[:], op0=OP.mult, op1=OP.add)
            ts_(pyb, ry[:], 1.0, 112.0, OP.add, OP.mult)

            t4 = wpool.tile([128, 128], F32)
            ts_(t4[:], x, Rc(5, b), None, OP.mult)
            t5 = wpool.tile([128, 128], F32)
            stt(t5[:], y, Rc(6, b), t4[:], op0=OP.mult, op1=OP.add)
            stt(rzb, z, Rc(7, b), t5[:], op0=OP.mult, op1=OP.add)

            nc.vector.tensor_reduce(zred[:, b:b + 1], rzb, axis=AX.X, op=OP.min)
            nc.vector.tensor_reduce(zred[:, 32 + b:32 + b + 1], rzb,
                                    axis=AX.X, op=OP.max)

        # ---------------- phase 1b: z min/max across partitions ----------------
        ztp = pspool2.tile([64, 128], F32, tag='ptmp')
        nc.tensor.transpose(out=ztp[:], in_=zred[:], identity=ident[:])
        zmm = cpool.tile([64, 1], F32)
        nc.vector.memset(zmm[:], 0.0)
        nc.vector.tensor_reduce(zmm[0:BPC, :], ztp[0:BPC, :], axis=AX.X, op=OP.min)
        nc.vector.tensor_reduce(zmm[32:32 + BPC, :], ztp[32:32 + BPC, :],
                                axis=AX.X, op=OP.max)
        zrp = pspool2.tile([1, 64], F32, tag='ptmp')
        nc.tensor.transpose(out=zrp[:], in_=zmm[:],
                            identity=ident[0:64, 0:64])
        zrow = cpool.tile([1, 64], F32)
        nc.vector.tensor_copy(zrow[:], zrp[:])
        zbp = pspool2.tile([128, 64], F32, tag='ptmp')
        nc.tensor.matmul(out=zbp[:], lhsT=ones_row[:], rhs=zrow[:],
                         start=True, stop=True)
        zbc = cpool.tile([128, 64], F32)
        nc.vector.tensor_copy(zbc[:], zbp[:])

        # feat = 0.3 + 0.7*(z - zmin)/(zmax - zmin + 1e-6) = z*inv07 + beta
        d_ = cpool.tile([128, BPC], F32)
        stt(d_[:], zbc[:, 32:32 + BPC], 1e-6, zbc[:, 0:BPC],
            op0=OP.add, op1=OP.subtract)
        rec = cpool.tile([128, BPC], F32)
        nc.vector.reciprocal(rec[:], d_[:])
        inv07 = cpool.tile([128, BPC], F32)
        ts_(inv07[:], rec[:], 0.7, None, OP.mult)
        tb = cpool.tile([128, BPC], F32)
        tt_(tb[:], zbc[:, 0:BPC], inv07[:], op=OP.mult)
        beta = cpool.tile([128, BPC], F32)
        ts_(beta[:], tb[:], -1.0, 0.3, OP.mult, OP.add)

        # ---------------- phase 2: hats + matmul per batch ----------------
        # Negation trick: build Atn = -f*hat_y and Btn = -hat_x; the two
        # negations cancel in the matmul, so no fixup is needed.
        #   y-side: u' = ACT Abs(j*f - f*py) = f*|j-py| ;
        #           Atn = min(u'-f, 0) = -f*hat_y              (1 ACT + 1 DVE)
        #   x-side (DVE path): pn = min(j-1-px, 0), qn = min(px-1-j, 0)
        #           (2-src ts from shifted iota consts);
        #           Btn = max(pn, qn) = -hat_x  (8-tile-wide TT max)
        #   x-side (ACT path, to balance engines): ux = ACT Abs(j - px);
        #           Btn = min(ux-1, 0) (imm-chain)
        TW = 8                    # tiles per wide group
        NYACT = 128               # y-tiles on the ACT path per batch
        BF16 = mybir.dt.bfloat16
        for b in range(BPC):
            pxE = px_all[:, b * 128:(b + 1) * 128]   # px + 0.5
            pyE = py_all[:, b * 128:(b + 1) * 128]
            rzb = rz_all[:, b * 128:(b + 1) * 128]

            feat = wpool.tile([128, 128], F32)
            ts_(feat[:], rzb, inv07[:, b:b + 1], beta[:, b:b + 1],
                OP.mult, OP.add)
            # mask: px>=0 & px<223 & py>=0 & py<223   (pxE = px+0.5)
            mx = wpool.tile([128, 128], F32)
            ts_(mx[:], pxE, 0.5, None, OP.is_ge)
            mx2 = wpool.tile([128, 128], F32)
            stt(mx2[:], pxE, 223.5, mx[:], op0=OP.is_lt, op1=OP.mult)
            my = wpool.tile([128, 128], F32)
            ts_(my[:], pyE, 0.5, None, OP.is_ge)
            my2 = wpool.tile([128, 128], F32)
            stt(my2[:], pyE, 223.5, my[:], op0=OP.is_lt, op1=OP.mult)
            fm = wpool.tile([128, 128], F32)
            tt_(fm[:], feat[:], mx2[:], op=OP.mult)
            featm = wpool.tile([128, 128], F32)
            tt_(featm[:], fm[:], my2[:], op=OP.mult)
            pym05 = wpool.tile([128, 128], F32)   # py
            ts_(pym05[:], pyE, 0.5, 0.0, OP.subtract, OP.add)
            pyneg = wpool.tile([128, 128], F32)   # -py
            ts_(pyneg[:], pyE, -1.0, 0.5, OP.mult, OP.add)
            # x-side scatter prep: x0 = floor(px) clamped to [0,222], weights
            # (frx-1, -frx) = negated bilinear x-weights
            # y-side scatter prep for the first SY tiles of each group:
            # weights (f*(fry-1), -f*fry) = negated f-scaled bilinear y-weights
            pyc = wpool.tile([128, 128], F32)
            ts_(pyc[:], pyE, 0.5, 222.99, OP.max, OP.min)
            pyf = wpool.tile([128, 128], F32)
            ts_(pyf[:], pyc[:], 0.5, None, OP.subtract)
            pyfm = wpool.tile([128, 128], F32)
            ts_(pyfm[:], pyf[:], 0.5, None, OP.subtract)
            yi0 = wpool.tile([128, 128], mybir.dt.int16)
            nc.vector.tensor_copy(yi0[:], pyfm[:])
            yi0f = wpool.tile([128, 128], F32)
            nc.vector.tensor_copy(yi0f[:], yi0[:])
            fry = wpool.tile([128, 128], F32)
            tt_(fry[:], pyf[:], yi0f[:], op=OP.subtract)
            wy1 = wpool.tile([128, 128], F32)
            stt(wy1[:], fry[:], 1.0, featm[:], op0=OP.subtract, op1=OP.mult)
            wy2 = wpool.tile([128, 128], F32)
            stt(wy2[:], fry[:], -1.0, featm[:], op0=OP.mult, op1=OP.mult)
            wypair = wpool.tile([128, 256], F16)
            wyv = wypair[:].rearrange("p (q t) -> p q t", t=2)
            nc.vector.tensor_copy(wyv[:, :, 0:1], wy1[:].unsqueeze(2))
            nc.vector.tensor_copy(wyv[:, :, 1:2], wy2[:].unsqueeze(2))
            pyf1 = wpool.tile([128, 128], F32)
            ts_(pyf1[:], pyf[:], 0.5, None, OP.add)
            yi1 = wpool.tile([128, 128], mybir.dt.int16)
            nc.vector.tensor_copy(yi1[:], pyf1[:])
            ypair = wpool.tile([128, 256], mybir.dt.int16)
            ypv = ypair[:].rearrange("p (q t) -> p q t", t=2)
            nc.vector.tensor_copy(ypv[:, :, 0:1], yi0[:].unsqueeze(2))
            nc.vector.tensor_copy(ypv[:, :, 1:2], yi1[:].unsqueeze(2))
            ypairO = wpool.tile([128, 256], mybir.dt.int16)
            tt_(ypairO[:], ypair[:], offp[:], op=OP.add)
            pxc = wpool.tile([128, 128], F32)
            ts_(pxc[:], pxE, 0.5, 222.99, OP.max, OP.min)
            pxf = wpool.tile([128, 128], F32)
            ts_(pxf[:], pxc[:], 0.5, None, OP.subtract)
            pxfm = wpool.tile([128, 128], F32)
            ts_(pxfm[:], pxf[:], 0.5, None, OP.subtract)
            xi0 = wpool.tile([128, 128], mybir.dt.int16)
            nc.vector.tensor_copy(xi0[:], pxfm[:])
            xi0f = wpool.tile([128, 128], F32)
            nc.vector.tensor_copy(xi0f[:], xi0[:])
            frx = wpool.tile([128, 128], F32)
            tt_(frx[:], pxf[:], xi0f[:], op=OP.subtract)
            w1n = wpool.tile([128, 128], F32)
            ts_(w1n[:], frx[:], 1.0, None, OP.subtract)
            w2n = wpool.tile([128, 128], F32)
            ts_(w2n[:], frx[:], -1.0, None, OP.mult)
            wpair = wpool.tile([128, 256], F16)
            wpv = wpair[:].rearrange("p (q t) -> p q t", t=2)
            nc.vector.tensor_copy(wpv[:, :, 0:1], w1n[:].unsqueeze(2))
            nc.vector.tensor_copy(wpv[:, :, 1:2], w2n[:].unsqueeze(2))
            pxf1 = wpool.tile([128, 128], F32)
            ts_(pxf1[:], pxf[:], 0.5, None, OP.add)
            xi1 = wpool.tile([128, 128], mybir.dt.int16)
            nc.vector.tensor_copy(xi1[:], pxf1[:])
            xpair = wpool.tile([128, 256], mybir.dt.int16)
            xpv = xpair[:].rearrange("p (q t) -> p q t", t=2)
            nc.vector.tensor_copy(xpv[:, :, 0:1], xi0[:].unsqueeze(2))
            nc.vector.tensor_copy(xpv[:, :, 1:2], xi1[:].unsqueeze(2))
            xpairO = wpool.tile([128, 256], mybir.dt.int16)
            tt_(xpairO[:], xpair[:], offp[:], op=OP.add)

            ps0 = pspool.tile([128, W], F32)
            ps1 = pspool.tile([128, W], F32)

            def ymm(q, btn_ap, at_ready=None):
                if at_ready is not None:
                    nc.tensor.matmul(out=ps0[:], lhsT=at_ready[:, 0:128],
                                     rhs=btn_ap,
                                     start=(q == 0), stop=(q == KT - 1))
                    nc.tensor.matmul(out=ps1[0:96, :],
                                     lhsT=at_ready[:, 128:224], rhs=btn_ap,
                                     start=(q == 0), stop=(q == KT - 1))
                    return
                At = hpool.tile([128, W], F16, tag="At")
                if q < KT - NYACT:
                    nc.vector._custom_dve(
                        NEGHAT, out=At[:], in0=iota_h[:],
                        s0=pym05[:, q:q + 1], s1=featm[:, q:q + 1], imm2=-1.0)
                else:
                    uy = hpool.tile([128, W], F16, tag="uy")
                    act(uy[:], iota_f[:], AF.Abs, bias=pyneg[:, q:q + 1])
                    nc.vector._custom_dve(
                        NEGHATP, out=At[:], in0=uy[:],
                        s0=0.0, s1=featm[:, q:q + 1], imm2=-1.0)
                nc.tensor.matmul(out=ps0[:], lhsT=At[:, 0:128], rhs=btn_ap,
                                 start=(q == 0), stop=(q == KT - 1))
                nc.tensor.matmul(out=ps1[0:96, :], lhsT=At[:, 128:224],
                                 rhs=btn_ap, start=(q == 0), stop=(q == KT - 1))

            SY = 4                    # y-tiles per group built by Pool scatter
            for g in range(KT // TW):
                q0 = g * TW
                btnw = hpool.tile([128, TW * W], F16, tag="btnw")
                nc.gpsimd.local_scatter(btnw[:], wpair[:, 2 * q0:2 * q0 + 16],
                                        xpairO[:, 2 * q0:2 * q0 + 16],
                                        channels=128,
                                        num_elems=TW * W, num_idxs=16)
                atw = hpool.tile([128, SY * W], F16, tag="atw")
                nc.gpsimd.local_scatter(atw[:], wypair[:, 2 * q0:2 * q0 + 2 * SY],
                                        ypairO[:, 2 * q0:2 * q0 + 2 * SY],
                                        channels=128,
                                        num_elems=SY * W, num_idxs=2 * SY)
                for j in range(TW):
                    at = atw[:, j * W:(j + 1) * W] if j < SY else None
                    ymm(q0 + j, btnw[:, j * W:(j + 1) * W], at_ready=at)

            out0 = wpool.tile([128, W], F32)
            nc.vector.tensor_copy(out0[:], ps0[:])
            out1 = wpool.tile([128, W], F32)
            nc.vector.tensor_copy(out1[0:96, :], ps1[0:96, :])
            nc.sync.dma_start(out=img_d[b, 0:128, :], in_=out0[:])
            nc.sync.dma_start(out=img_d[b, 128:224, :], in_=out1[0:96, :])


@functools.lru_cache(maxsize=1)
def _get_compiled():
    nc = bacc.Bacc(
        "TRN2",
        target_bir_lowering=False,
        debug=False,
        enable_asserts=False,
        num_devices=NCORES,
    )
    pts_d = nc.dram_tensor("points", [BPC, N, 3], F32, kind="ExternalInput")
    az_d = nc.dram_tensor("azimuth", [BPC], F32, kind="ExternalInput")
    el_d = nc.dram_tensor("elevation", [BPC], F32, kind="ExternalInput")
    img_d = nc.dram_tensor("img", [BPC, H, W], F32, kind="ExternalOutput")
    with tile.TileContext(nc) as tc:
        splat_kernel(tc, nc, pts_d, az_d, el_d, img_d)
    nc.compile()
    return nc


def run_on_device(points, azimuth, elevation, trace=False, **kw):
    nc = _get_compiled()
    in_maps = []
    for i in range(NCORES):
        s = slice(i * BPC, (i + 1) * BPC)
        in_maps.append({
            "points": np.ascontiguousarray(points[s], dtype=np.float32),
            "azimuth": np.ascontiguousarray(azimuth[s], dtype=np.float32),
            "elevation": np.ascontiguousarray(elevation[s], dtype=np.float32),
        })
    return run_bass_kernel_spmd(nc, in_maps, list(range(NCORES)),
                                trace=trace, **kw)


def kernel(points, azimuth, elevation):
    res = run_on_device(points, azimuth, elevation)
    imgs = np.concatenate([res.results[i]["img"] for i in range(NCORES)], axis=0)
    out = np.empty((B, 3, H, W), dtype=np.float32)
    out[:] = imgs[:, None, :, :]
    return out

